# revision 11
# baseline (speedup 1.0000x reference)
"""Trainium2 Bass kernel for the rank-1-logit attention module (8 NeuronCores).

Reference computation (per batch b of 2, head n of 12, feature d of 64):
    qkv = w_qkv @ x                                  (1x1 conv, c=256 -> 2304)
    logits[i,j] = q_i * k_j * (1/8)                  (rank-1 outer product, hw=256)
    attn = softmax_j(logits);  out_i = sum_j attn[i,j] v_j
    y = InstanceNorm(x + w_out @ out + b_out)

Key algebraic optimization: because logits are rank-1 in the exponent and
|q_i*k_j/8| <= ~0.34, exp() is replaced by a short Taylor series, which
collapses the (hw x hw) softmax per (b,n,d) into M+1 scalar moments:
    num(i) = sum_m KV_m q_i^m,  den(i) = sum_m G_m q_i^m,  out_i = num/den
    with  P_m[j] = (k_j/8)^m/m!,  G_m = sum_j P_m[j],  KV_m = sum_j P_m[j] v_j
Truncation error at M=6 is ~1e-7 relative (validated in numpy), far below
the f32 rounding floor of the pipeline.

Sharding: 8 cores x 3 (batch,head) pairs each (cores 0-3: batch 0, 4-7:
batch 1).  Each core computes QKV + moment-attention for its 192 rows
(3 heads x 64 features), the partial output projection for its batch, then a
4-core ReduceScatter sums projections and leaves each core a 64-channel
slice on which it applies residual + bias + InstanceNorm.
"""

import numpy as np

import concourse.bacc as bacc
import concourse.bass as bass
import concourse.mybir as mybir
import concourse.tile as tile
from concourse.bass_utils import run_bass_kernel_spmd

B, C, H, W = 2, 256, 16, 16
HW = H * W  # 256
NH, D = 12, 64  # heads, head features
SCALE = float(D) ** -0.5  # 1/8
EPS = 1e-5
NCORES = 8
PAIRS = 3  # (b, n) pairs per core
R = PAIRS * D  # 192 qkv rows per core
M = 6  # Taylor order
FP = mybir.dt.float32

_cache = {}


def _build(stage=9):
    nc = bacc.Bacc("TRN2", target_bir_lowering=False, debug=False, num_devices=NCORES)

    wq_d = nc.dram_tensor("wq_lhsT", [C, 3 * R], FP, kind="ExternalInput")
    x_d = nc.dram_tensor("xb", [C, HW], FP, kind="ExternalInput")
    wo_d = nc.dram_tensor("wo_lhsT", [R, C], FP, kind="ExternalInput")
    xsl_d = nc.dram_tensor("x_sl", [64, HW], FP, kind="ExternalInput")
    bout_d = nc.dram_tensor("bout_sl", [64, 1], FP, kind="ExternalInput")
    out_d = nc.dram_tensor("out", [64, HW], FP, kind="ExternalOutput")

    RG = [[0, 1, 2, 3], [4, 5, 6, 7]]
    AX = mybir.AluOpType
    AF = mybir.ActivationFunctionType
    X = mybir.AxisListType.X

    with tile.TileContext(nc) as tc:
        with (
            tc.tile_pool(name="sb", bufs=1) as sb,
            tc.tile_pool(name="ps", bufs=1, space="PSUM") as ps,
            tc.tile_pool(name="dram", bufs=1, space="DRAM") as dram,
        ):
            # ---- loads ----
            wq_sb = sb.tile([128, 2, 3 * R], FP, tag="wq")
            nc.sync.dma_start(wq_sb[:], wq_d.rearrange("(a p) m -> p a m", p=128))
            x_sb = sb.tile([128, 2, HW], FP, tag="x")
            nc.sync.dma_start(x_sb[:], x_d.rearrange("(a p) j -> p a j", p=128))
            wo_sb = sb.tile([128, 2, C], FP, tag="wo")
            nc.sync.dma_start(wo_sb[:, 0, :], wo_d[0:128, :])
            nc.sync.dma_start(wo_sb[0:64, 1, :], wo_d[128:R, :])
            xsl_sb = sb.tile([64, HW], FP, tag="xsl")
            nc.sync.dma_start(xsl_sb[:], xsl_d[:])
            bout_sb = sb.tile([64, 1], FP, tag="bout")
            nc.sync.dma_start(bout_sb[:], bout_d[:])

            # ---- qkv projection: 192 rows each of K, V, Q ----
            # lhsT columns: [K 0:192 | V 192:384 | Q 384:576], each block in
            # (pair, d) order; chunked into M-slices of (128, 64) rows.
            psK = ps.tile([128, 2, HW], FP, tag="psK")
            psV = ps.tile([128, 2, HW], FP, tag="psV")
            psQ = ps.tile([128, 2, HW], FP, tag="psQ")
            mslices = [
                (0, psK, 0, 128), (128, psK, 1, 64),
                (192, psV, 0, 128), (320, psV, 1, 64),
                (384, psQ, 0, 128), (512, psQ, 1, 64),
            ]
            for col, pst, ci, rows in mslices:
                for a in range(2):
                    nc.tensor.matmul(
                        pst[0:rows, ci, :],
                        wq_sb[:, a, col:col + rows],
                        x_sb[:, a, :],
                        start=(a == 0),
                        stop=(a == 1),
                    )

            k_sb = sb.tile([128, 2, HW], FP, tag="k")
            v_sb = sb.tile([128, 2, HW], FP, tag="v")
            q_sb = sb.tile([128, 2, HW], FP, tag="q")
            for t, pst in ((k_sb, psK), (v_sb, psV), (q_sb, psQ)):
                nc.vector.tensor_copy(t[:, 0, :], pst[:, 0, :])
                nc.vector.tensor_copy(t[0:64, 1, :], pst[0:64, 1, :])

            if stage == 1:
                nc.sync.dma_start(out_d[:], k_sb[0:64, 0, :])

            # ---- moments + Horner per row-chunk ----
            attn = sb.tile([128, 2, HW], FP, tag="attn")
            for ci, rows in ((0, 128), (1, 64)) if stage >= 2 else ():
                Kc = k_sb[0:rows, ci, :]
                Vc = v_sb[0:rows, ci, :]
                Qc = q_sb[0:rows, ci, :]

                G = sb.tile([128, M + 1], FP, tag=f"G{ci}")
                KV = sb.tile([128, M + 1], FP, tag=f"KV{ci}")
                nc.vector.memset(G[0:rows, 0:1], float(HW))
                nc.vector.tensor_reduce(KV[0:rows, 0:1], Vc, axis=X, op=AX.add)

                Pprev = None
                for m in range(1, M + 1):
                    Pm = sb.tile([128, HW], FP, tag=f"P{ci}_{m}")
                    PV = sb.tile([128, HW], FP, tag=f"PV{ci}_{m}")
                    if m == 1:
                        nc.vector.tensor_scalar(
                            Pm[0:rows], Kc, SCALE, None, AX.mult, AX.add,
                            accum_out=G[0:rows, 1:2],
                        )
                    else:
                        nc.vector.scalar_tensor_tensor(
                            Pm[0:rows], Pprev[0:rows], SCALE / m, Kc,
                            AX.mult, AX.mult,
                            accum_out=G[0:rows, m:m + 1],
                        )
                    nc.vector.scalar_tensor_tensor(
                        PV[0:rows], Pm[0:rows], 1.0, Vc,
                        AX.mult, AX.mult,
                        accum_out=KV[0:rows, m:m + 1],
                    )
                    Pprev = Pm

                # Horner in q for numerator (KV coeffs) and denominator (G)
                polys = []
                for pi, Cf in ((0, KV), (1, G)):
                    acc = sb.tile([128, HW], FP, tag=f"h{ci}_{pi}_a")
                    nc.vector.tensor_scalar(
                        acc[0:rows], Qc,
                        Cf[0:rows, M:M + 1], Cf[0:rows, M - 1:M],
                        AX.mult, AX.add,
                    )
                    for m in range(M - 2, -1, -1):
                        t2 = sb.tile([128, HW], FP, tag=f"h{ci}_{pi}_m{m}")
                        nc.vector.tensor_mul(t2[0:rows], acc[0:rows], Qc)
                        acc = sb.tile([128, HW], FP, tag=f"h{ci}_{pi}_s{m}")
                        nc.scalar.activation(
                            acc[0:rows], t2[0:rows], AF.Identity,
                            bias=Cf[0:rows, m:m + 1],
                        )
                    polys.append(acc)

                num, den = polys
                rden = sb.tile([128, HW], FP, tag=f"rden{ci}")
                nc.vector.reciprocal(rden[0:rows], den[0:rows])
                nc.vector.tensor_mul(attn[0:rows, ci, :], num[0:rows], rden[0:rows])

            if stage == 2:
                nc.sync.dma_start(out_d[:], attn[0:64, 0, :])

            if stage >= 3:
                # ---- output projection (partial over this core's channels) ----
                psY = ps.tile([128, 2, HW], FP, tag="psY")
                for mc in range(2):
                    nc.tensor.matmul(
                        psY[:, mc, :], wo_sb[:, 0, mc * 128:(mc + 1) * 128],
                        attn[:, 0, :], start=True, stop=False,
                    )
                    nc.tensor.matmul(
                        psY[:, mc, :], wo_sb[0:64, 1, mc * 128:(mc + 1) * 128],
                        attn[0:64, 1, :], start=False, stop=True,
                    )

                if stage == 3:
                    ysb3 = sb.tile([64, HW], FP, tag="ysb3")
                    nc.vector.tensor_copy(ysb3[:], psY[0:64, 0, :])
                    nc.sync.dma_start(out_d[:], ysb3[:])

            if stage >= 4:
                # ---- ReduceScatter partials within each batch group ----
                ysb = sb.tile([128, 2, HW], FP, tag="ysb")
                nc.vector.tensor_copy(ysb[:, 0, :], psY[:, 0, :])
                nc.vector.tensor_copy(ysb[:, 1, :], psY[:, 1, :])
                rs_in = dram.tile([C, HW], FP, tag="rs_in")
                nc.sync.dma_start(rs_in[0:128, :], ysb[:, 0, :])
                nc.sync.dma_start(rs_in[128:C, :], ysb[:, 1, :])
                rs_out = dram.tile([64, HW], FP, tag="rs_out")
                nc.gpsimd.collective_compute(
                    "ReduceScatter", AX.add, replica_groups=RG,
                    ins=[rs_in[:].opt()], outs=[rs_out[:].opt()],
                )

                if stage == 4:
                    y04 = sb.tile([64, HW], FP, tag="y04")
                    nc.sync.dma_start(y04[:], rs_out[:])
                    nc.sync.dma_start(out_d[:], y04[:])

            if stage >= 5:
                # ---- residual + bias + InstanceNorm on 64-channel slice ----
                y0 = sb.tile([64, HW], FP, tag="y0")
                nc.sync.dma_start(y0[:], rs_out[:])
                y1 = sb.tile([64, HW], FP, tag="y1")
                nc.vector.tensor_add(y1[:], y0[:], xsl_sb[:])
                y = sb.tile([64, HW], FP, tag="y")
                nc.scalar.activation(y[:], y1[:], AF.Identity, bias=bout_sb[:, 0:1])

                musum = sb.tile([64, 1], FP, tag="musum")
                nc.vector.tensor_reduce(musum[:], y[:], axis=X, op=AX.add)
                negmu = sb.tile([64, 1], FP, tag="negmu")
                nc.scalar.activation(negmu[:], musum[:], AF.Copy, scale=-1.0 / HW)
                sq = sb.tile([64, HW], FP, tag="sq")
                varsum = sb.tile([64, 1], FP, tag="varsum")
                nc.scalar.activation(
                    sq[:], y[:], AF.Square, bias=negmu[:, 0:1], accum_out=varsum[:],
                )
                epsv = sb.tile([64, 1], FP, tag="epsv")
                nc.vector.memset(epsv[:], EPS)
                stds = sb.tile([64, 1], FP, tag="stds")
                nc.scalar.activation(
                    stds[:], varsum[:], AF.Sqrt, scale=1.0 / HW, bias=epsv[:, 0:1],
                )
                rstd = sb.tile([64, 1], FP, tag="rstd")
                nc.vector.reciprocal(rstd[:], stds[:])
                nmr = sb.tile([64, 1], FP, tag="nmr")
                nc.vector.tensor_mul(nmr[:], negmu[:], rstd[:])

                out_sb = sb.tile([64, HW], FP, tag="outsb")
                nc.scalar.activation(
                    out_sb[:], y[:], AF.Identity,
                    scale=rstd[:, 0:1], bias=nmr[:, 0:1],
                )
                nc.sync.dma_start(out_d[:], out_sb[:])

    nc.compile()
    return nc


def _shard_inputs(x, w_qkv, w_out, b_out):
    x = np.ascontiguousarray(x, dtype=np.float32)
    w_qkv = np.ascontiguousarray(w_qkv, dtype=np.float32)
    w_out = np.ascontiguousarray(w_out, dtype=np.float32)
    b_out = np.ascontiguousarray(b_out, dtype=np.float32)
    xf = x.reshape(B, C, HW)
    in_maps = []
    for g in range(NCORES):
        bg = g // 4
        heads = [3 * (g % 4) + i for i in range(PAIRS)]
        ks = np.concatenate([np.arange(D) + 768 + n * D for n in heads])
        vs = np.concatenate([np.arange(D) + 1536 + n * D for n in heads])
        qs = np.concatenate([np.arange(D) + n * D for n in heads])
        wq_lhsT = np.ascontiguousarray(w_qkv[np.concatenate([ks, vs, qs]), :].T)
        o_chan = np.concatenate([np.arange(D) + n * D for n in heads])
        wo_lhsT = np.ascontiguousarray(w_out[:, o_chan].T)
        csl = slice(64 * (g % 4), 64 * (g % 4) + 64)
        in_maps.append({
            "wq_lhsT": wq_lhsT,
            "xb": xf[bg],
            "wo_lhsT": wo_lhsT,
            "x_sl": np.ascontiguousarray(xf[bg, csl]),
            "bout_sl": np.ascontiguousarray(b_out[csl]).reshape(64, 1),
        })
    return in_maps


def kernel(x, w_qkv, w_out, b_out, _trace=False, _trace_kwargs=None):
    if "nc" not in _cache:
        _cache["nc"] = _build()
    nc = _cache["nc"]
    in_maps = _shard_inputs(x, w_qkv, w_out, b_out)
    res = run_bass_kernel_spmd(
        nc, in_maps, core_ids=list(range(NCORES)),
        trace=_trace, **(_trace_kwargs or {}),
    )
    _cache["last_result"] = res
    out = np.empty((B, C, HW), np.float32)
    for g in range(NCORES):
        bg = g // 4
        csl = slice(64 * (g % 4), 64 * (g % 4) + 64)
        out[bg, csl] = res.results[g]["out"]
    return out.reshape(B, C, H, W)


# revision 13
# speedup vs baseline: 1.1197x; 1.1197x over previous
"""Trainium2 Bass kernel for the rank-1-logit attention module (8 NeuronCores).

Reference computation (per batch b of 2, head n of 12, feature d of 64):
    qkv = w_qkv @ x                                  (1x1 conv, c=256 -> 2304)
    logits[i,j] = q_i * k_j * (1/8)                  (rank-1 outer product, hw=256)
    attn = softmax_j(logits);  out_i = sum_j attn[i,j] v_j
    y = InstanceNorm(x + w_out @ out + b_out)

Key algebraic optimization: because logits are rank-1 in the exponent and
|q_i*k_j/8| <= ~0.34, exp() is replaced by a short Taylor series, which
collapses the (hw x hw) softmax per (b,n,d) into M+1 scalar moments:
    num(i) = sum_m KV_m q_i^m,  den(i) = sum_m G_m q_i^m,  out_i = num/den
    with  P_m[j] = (k_j/8)^m/m!,  G_m = sum_j P_m[j],  KV_m = sum_j P_m[j] v_j
Truncation error at M=5 is ~5e-6 relative (validated in numpy), far below
the 2e-2 gate; bf16 matmul inputs add ~2e-5.

Sharding: 8 cores x 3 (batch,head) pairs each (cores 0-3: batch 0, 4-7:
batch 1).  Each core computes QKV + moment-attention for its 192 rows
(3 heads x 64 features), the partial output projection for its batch, then a
4-core ReduceScatter (bf16 payload) sums projections and leaves each core a
64-channel slice on which it applies residual + bias + InstanceNorm.
"""

import numpy as np
import ml_dtypes

import concourse.bacc as bacc
import concourse.bass as bass
import concourse.mybir as mybir
import concourse.tile as tile
from concourse.bass_utils import run_bass_kernel_spmd

B, C, H, W = 2, 256, 16, 16
HW = H * W  # 256
NH, D = 12, 64  # heads, head features
SCALE = float(D) ** -0.5  # 1/8
EPS = 1e-5
NCORES = 8
PAIRS = 3  # (b, n) pairs per core
R = PAIRS * D  # 192 qkv rows per core
M = 5  # Taylor order
FP = mybir.dt.float32
BF = mybir.dt.bfloat16

_cache = {}


def _build(stage=9):
    nc = bacc.Bacc("TRN2", target_bir_lowering=False, debug=False, num_devices=NCORES)

    wq_d = nc.dram_tensor("wq_lhsT", [C, 3 * R], BF, kind="ExternalInput")
    x_d = nc.dram_tensor("xb", [C, HW], BF, kind="ExternalInput")
    wo_d = nc.dram_tensor("wo_lhsT", [R, C], BF, kind="ExternalInput")
    xsl_d = nc.dram_tensor("x_sl", [64, HW], FP, kind="ExternalInput")
    bout_d = nc.dram_tensor("bout_sl", [64, 1], FP, kind="ExternalInput")
    out_d = nc.dram_tensor("out", [64, HW], FP, kind="ExternalOutput")

    RG = [[0, 1, 2, 3], [4, 5, 6, 7]]
    AX = mybir.AluOpType
    AF = mybir.ActivationFunctionType
    X = mybir.AxisListType.X

    with tile.TileContext(nc) as tc:
        with (
            tc.tile_pool(name="sb", bufs=1) as sb,
            tc.tile_pool(name="ps", bufs=1, space="PSUM") as ps,
            tc.tile_pool(name="dram", bufs=1, space="DRAM") as dram,
        ):
            # ---- loads (spread across engine DMA queues) ----
            wq_sb = sb.tile([128, 2, 3 * R], BF, tag="wq")
            nc.scalar.dma_start(wq_sb[:], wq_d.rearrange("(a p) m -> p a m", p=128))
            x_sb = sb.tile([128, 2, HW], BF, tag="x")
            nc.sync.dma_start(x_sb[:], x_d.rearrange("(a p) j -> p a j", p=128))
            wo_sb = sb.tile([128, 2, C], BF, tag="wo")
            nc.gpsimd.dma_start(wo_sb[:, 0, :], wo_d[0:128, :])
            nc.gpsimd.dma_start(wo_sb[0:64, 1, :], wo_d[128:R, :])
            xsl_sb = sb.tile([64, HW], FP, tag="xsl")
            nc.gpsimd.dma_start(xsl_sb[:], xsl_d[:])
            bout_sb = sb.tile([64, 1], FP, tag="bout")
            nc.gpsimd.dma_start(bout_sb[:], bout_d[:])

            # ---- qkv projection: 192 rows each of K, V, Q ----
            # lhsT columns: [K 0:192 | V 192:384 | Q 384:576], each block in
            # (pair, d) order; chunked into M-slices of (128, 64) rows.
            psK = ps.tile([128, 2, HW], FP, tag="psK")
            psV = ps.tile([128, 2, HW], FP, tag="psV")
            psQ = ps.tile([128, 2, HW], FP, tag="psQ")
            mslices = [
                (0, psK, 0, 128), (128, psK, 1, 64),
                (192, psV, 0, 128), (320, psV, 1, 64),
                (384, psQ, 0, 128), (512, psQ, 1, 64),
            ]
            for col, pst, ci, rows in mslices:
                for a in range(2):
                    nc.tensor.matmul(
                        pst[0:rows, ci, :],
                        wq_sb[:, a, col:col + rows],
                        x_sb[:, a, :],
                        start=(a == 0),
                        stop=(a == 1),
                    )

            if stage == 1:
                o1 = sb.tile([64, HW], FP, tag="o1")
                nc.vector.tensor_copy(o1[:], psK[0:64, 0, :])
                nc.sync.dma_start(out_d[:], o1[:])

            # ---- moments + Horner per row-chunk (direct from PSUM) ----
            attn = sb.tile([128, 2, HW], BF, tag="attn")
            for ci, rows in ((0, 128), (1, 64)) if stage >= 2 else ():
                Kc = psK[0:rows, ci, :]
                Vc = psV[0:rows, ci, :]
                Qc = psQ[0:rows, ci, :]

                G = sb.tile([128, M + 1], FP, tag=f"G{ci}")
                KV = sb.tile([128, M + 1], FP, tag=f"KV{ci}")
                nc.vector.memset(G[0:rows, 0:1], float(HW))
                nc.vector.tensor_reduce(KV[0:rows, 0:1], Vc, axis=X, op=AX.add)

                Pprev = None
                for m in range(1, M + 1):
                    Pm = sb.tile([128, HW], FP, tag=f"P{ci}_{m}")
                    PV = sb.tile([128, HW], FP, tag=f"PV{ci}_{m}")
                    if m == 1:
                        nc.vector.tensor_scalar(
                            Pm[0:rows], Kc, SCALE, None, AX.mult, AX.add,
                            accum_out=G[0:rows, 1:2],
                        )
                    else:
                        nc.vector.scalar_tensor_tensor(
                            Pm[0:rows], Pprev[0:rows], SCALE / m, Kc,
                            AX.mult, AX.mult,
                            accum_out=G[0:rows, m:m + 1],
                        )
                    nc.vector.scalar_tensor_tensor(
                        PV[0:rows], Pm[0:rows], 1.0, Vc,
                        AX.mult, AX.mult,
                        accum_out=KV[0:rows, m:m + 1],
                    )
                    Pprev = Pm

                # Horner in q for numerator (KV coeffs) and denominator (G)
                polys = []
                for pi, Cf in ((0, KV), (1, G)):
                    acc = sb.tile([128, HW], FP, tag=f"h{ci}_{pi}_a")
                    nc.vector.tensor_scalar(
                        acc[0:rows], Qc,
                        Cf[0:rows, M:M + 1], Cf[0:rows, M - 1:M],
                        AX.mult, AX.add,
                    )
                    for m in range(M - 2, -1, -1):
                        t2 = sb.tile([128, HW], FP, tag=f"h{ci}_{pi}_m{m}")
                        nc.vector.tensor_mul(t2[0:rows], acc[0:rows], Qc)
                        acc = sb.tile([128, HW], FP, tag=f"h{ci}_{pi}_s{m}")
                        nc.scalar.activation(
                            acc[0:rows], t2[0:rows], AF.Identity,
                            bias=Cf[0:rows, m:m + 1],
                        )
                    polys.append(acc)

                num, den = polys
                rden = sb.tile([128, HW], FP, tag=f"rden{ci}")
                rscr = sb.tile([128, HW], FP, tag=f"rscr{ci}")
                nc.vector.reciprocal_approx_accurate(
                    out=rden[0:rows], in_=den[0:rows], scratch=rscr[0:rows],
                )
                nc.vector.tensor_mul(attn[0:rows, ci, :], num[0:rows], rden[0:rows])

            if stage == 2:
                o2 = sb.tile([64, HW], FP, tag="o2")
                nc.vector.tensor_copy(o2[:], attn[0:64, 0, :])
                nc.sync.dma_start(out_d[:], o2[:])

            if stage >= 3:
                # ---- output projection (partial over this core's channels) ----
                psY = ps.tile([128, 2, HW], FP, tag="psY")
                for mc in range(2):
                    nc.tensor.matmul(
                        psY[:, mc, :], wo_sb[:, 0, mc * 128:(mc + 1) * 128],
                        attn[:, 0, :], start=True, stop=False,
                    )
                    nc.tensor.matmul(
                        psY[:, mc, :], wo_sb[0:64, 1, mc * 128:(mc + 1) * 128],
                        attn[0:64, 1, :], start=False, stop=True,
                    )

            if stage >= 4:
                # ---- ReduceScatter partials (bf16) within each batch group ----
                ysb = sb.tile([128, 2, HW], BF, tag="ysb")
                nc.vector.tensor_copy(ysb[:, 0, :], psY[:, 0, :])
                nc.vector.tensor_copy(ysb[:, 1, :], psY[:, 1, :])
                rs_in = dram.tile([C, HW], BF, tag="rs_in")
                nc.sync.dma_start(rs_in[0:128, :], ysb[:, 0, :])
                nc.sync.dma_start(rs_in[128:C, :], ysb[:, 1, :])
                rs_out = dram.tile([64, HW], BF, tag="rs_out")
                nc.gpsimd.collective_compute(
                    "ReduceScatter", AX.add, replica_groups=RG,
                    ins=[rs_in[:].opt()], outs=[rs_out[:].opt()],
                )

            if stage >= 5:
                # ---- residual + bias + InstanceNorm on 64-channel slice ----
                y0 = sb.tile([64, HW], BF, tag="y0")
                nc.sync.dma_start(y0[:], rs_out[:])
                y = sb.tile([64, HW], FP, tag="y")
                musum = sb.tile([64, 1], FP, tag="musum")
                # y = (rs + b_out) + x_sl ; musum = sum_j y
                nc.vector.scalar_tensor_tensor(
                    y[:], y0[:], bout_sb[:, 0:1], xsl_sb[:],
                    AX.add, AX.add, accum_out=musum[:],
                )
                ysq = sb.tile([64, HW], FP, tag="ysq")
                sqsum = sb.tile([64, 1], FP, tag="sqsum")
                nc.vector.scalar_tensor_tensor(
                    ysq[:], y[:], 1.0, y[:],
                    AX.mult, AX.mult, accum_out=sqsum[:],
                )
                negmu = sb.tile([64, 1], FP, tag="negmu")
                nc.vector.tensor_scalar(negmu[:], musum[:], -1.0 / HW, None, AX.mult)
                m2 = sb.tile([64, 1], FP, tag="m2")
                nc.vector.tensor_scalar(
                    m2[:], musum[:], musum[:, 0:1], 1.0 / (HW * HW), AX.mult, AX.mult,
                )
                t1 = sb.tile([64, 1], FP, tag="t1")
                nc.vector.tensor_scalar(t1[:], sqsum[:], 1.0 / HW, EPS, AX.mult, AX.add)
                vr = sb.tile([64, 1], FP, tag="vr")
                nc.vector.tensor_sub(vr[:], t1[:], m2[:])
                stds = sb.tile([64, 1], FP, tag="stds")
                nc.scalar.activation(stds[:], vr[:], AF.Sqrt)
                rstd = sb.tile([64, 1], FP, tag="rstd")
                nc.vector.reciprocal(rstd[:], stds[:])
                nmr = sb.tile([64, 1], FP, tag="nmr")
                nc.vector.tensor_mul(nmr[:], negmu[:], rstd[:])

                out_sb = sb.tile([64, HW], FP, tag="outsb")
                nc.vector.tensor_scalar(
                    out_sb[:], y[:], rstd[:, 0:1], nmr[:, 0:1], AX.mult, AX.add,
                )
                nc.sync.dma_start(out_d[:], out_sb[:])

    nc.compile()
    return nc


def _shard_inputs(x, w_qkv, w_out, b_out):
    x = np.ascontiguousarray(x, dtype=np.float32)
    w_qkv = np.ascontiguousarray(w_qkv, dtype=np.float32)
    w_out = np.ascontiguousarray(w_out, dtype=np.float32)
    b_out = np.ascontiguousarray(b_out, dtype=np.float32)
    bf16 = ml_dtypes.bfloat16
    xf = x.reshape(B, C, HW)
    in_maps = []
    for g in range(NCORES):
        bg = g // 4
        heads = [3 * (g % 4) + i for i in range(PAIRS)]
        ks = np.concatenate([np.arange(D) + 768 + n * D for n in heads])
        vs = np.concatenate([np.arange(D) + 1536 + n * D for n in heads])
        qs = np.concatenate([np.arange(D) + n * D for n in heads])
        wq_lhsT = np.ascontiguousarray(w_qkv[np.concatenate([ks, vs, qs]), :].T)
        o_chan = np.concatenate([np.arange(D) + n * D for n in heads])
        wo_lhsT = np.ascontiguousarray(w_out[:, o_chan].T)
        csl = slice(64 * (g % 4), 64 * (g % 4) + 64)
        in_maps.append({
            "wq_lhsT": wq_lhsT.astype(bf16),
            "xb": np.ascontiguousarray(xf[bg]).astype(bf16),
            "wo_lhsT": wo_lhsT.astype(bf16),
            "x_sl": np.ascontiguousarray(xf[bg, csl]),
            "bout_sl": np.ascontiguousarray(b_out[csl]).reshape(64, 1),
        })
    return in_maps


def kernel(x, w_qkv, w_out, b_out, _trace=False, _trace_kwargs=None):
    if "nc" not in _cache:
        _cache["nc"] = _build()
    nc = _cache["nc"]
    in_maps = _shard_inputs(x, w_qkv, w_out, b_out)
    res = run_bass_kernel_spmd(
        nc, in_maps, core_ids=list(range(NCORES)),
        trace=_trace, **(_trace_kwargs or {}),
    )
    _cache["last_result"] = res
    out = np.empty((B, C, HW), np.float32)
    for g in range(NCORES):
        bg = g // 4
        csl = slice(64 * (g % 4), 64 * (g % 4) + 64)
        out[bg, csl] = res.results[g]["out"]
    return out.reshape(B, C, H, W)
